# revision 6
# baseline (speedup 1.0000x reference)
"""Trainium2 Bass kernel for nn_Attension_Point (PointNet++-style grouping +
non-local attention + classifier head).

Sharding: pure data parallel — batch 32 split as 4 samples per NeuronCore
across 8 cores; all parameters replicated. Each core runs an identical NEFF
(SPMD) taking [4, ...] inputs and producing [60, 4] logits; the host
reassembles [32, 60].

Self-contained: hardcodes all shapes for the fixed problem size
B=32, D_IN=6, N1=512, K=64, N2=128, NUM_CLASS=60.
"""
import os
import numpy as np
import ml_dtypes

B, D_IN, N1, K, N2, NUM_CLASS = 32, 6, 512, 64, 128, 60
NCORE = 8
SPC = B // NCORE          # samples per core = 4
RADIUS2 = 0.5
POS = N1 * K              # 32768 positions per sample
NBLK = POS // 512         # 64 blocks of 512 positions (8 n-points each)

BF = ml_dtypes.bfloat16

_built = None


def _to_bf(x):
    return np.ascontiguousarray(np.asarray(x, np.float32).astype(BF))


def _to_f32(x):
    return np.ascontiguousarray(np.asarray(x, np.float32))


def _prep_weights(params):
    """Host-side parameter preparation into device layouts."""
    p = params
    w = {}

    def blk(b):
        return tuple(np.asarray(a, np.float32) for a in b)

    # ---- tower1 ----
    (W1, b1, g1, be1), (W2, b2, g2, be2), (W3, b3, g3, be3) = [blk(x) for x in p['netR_1']]
    t1w1 = np.zeros((128, 128), np.float32)
    for g in range(4):
        t1w1[32 * g:32 * g + 6, 0:64] = W1.T
        t1w1[32 * g + 6:32 * g + 12, 64:128] = W1.T
    w['t1w1'] = _to_f32(t1w1)
    w['t1s1'] = _to_f32(np.tile(g1, 2).reshape(128, 1))
    w['t1b1'] = _to_f32(np.tile(g1 * b1 + be1, 2).reshape(128, 1))
    t1w2 = np.zeros((128, 128), np.float32)
    t1w2[0:64, 0:64] = W2.T
    t1w2[64:128, 64:128] = W2.T
    w['t1w2'] = _to_bf(t1w2)
    w['t1s2'] = _to_f32(np.tile(g2, 2).reshape(128, 1))
    w['t1b2'] = _to_f32(np.tile(g2 * b2 + be2, 2).reshape(128, 1))
    t1w3 = np.zeros((128, 128), np.float32)
    t1w3[0:64, :] = W3.T
    t1w3[64:128, :] = W3.T
    w['t1w3'] = _to_bf(t1w3)
    w['t1s3'] = _to_f32(g3.reshape(128, 1))
    w['t1b3'] = _to_f32((g3 * b3 + be3).reshape(128, 1))

    # ---- tower2 (netR_2): input ch order [xyz(3); feat(128)] ----
    (V1, c1, h1, he1), (V2, c2, h2, he2), (V3, c3, h3, he3) = [blk(x) for x in p['netR_2']]
    V1s = h1[:, None] * V1            # fold bn scale into weights
    w['w2fa'] = _to_bf(V1s[:, 3:].T)          # [128, 128] feat part (lhsT)
    w['w2fb'] = _to_bf(V1s[:, :3].T)          # [3, 128] xyz part
    w['w2cn'] = _to_bf(-V1s[:, :3].T)         # [3, 128] for WCneg
    w['b2l1'] = _to_f32((h1 * c1 + he1).reshape(128, 1))
    w['w2l2'] = _to_bf(V2.T)                   # [128, 128]
    w['s2l2'] = _to_f32(h2.reshape(128, 1))
    w['b2l2'] = _to_f32((h2 * c2 + he2).reshape(128, 1))
    w['w2l3a'] = _to_bf(V3[0:128, :].T)        # [128, 128]
    w['w2l3b'] = _to_bf(V3[128:256, :].T)
    w['s2l3a'] = _to_f32(h3[0:128].reshape(128, 1))
    w['b2l3a'] = _to_f32((h3 * c3 + he3)[0:128].reshape(128, 1))
    w['s2l3b'] = _to_f32(h3[128:256].reshape(128, 1))
    w['b2l3b'] = _to_f32((h3 * c3 + he3)[128:256].reshape(128, 1))

    # ---- attention ----
    nl = {k: np.asarray(v, np.float32) for k, v in p['nl'].items()}
    # x chunks along contraction c: A = ch 3:131, B = ch 131:259, c = ch 0:3 (+ones)
    for nm in ('th', 'ph'):
        W = nl['W' + nm]          # [129, 259]
        bias = nl['b' + nm]       # [129]
        WT = W.T                  # [259, 129]
        w[f'w{nm}_A0'] = _to_bf(WT[3:131, 0:128])
        w[f'w{nm}_B0'] = _to_bf(WT[131:259, 0:128])
        c0 = np.zeros((4, 128), np.float32)
        c0[0:3, :] = WT[0:3, 0:128]
        c0[3, :] = bias[0:128]
        w[f'w{nm}_c0'] = _to_bf(c0)
        w[f'w{nm}_A1'] = _to_bf(WT[3:131, 128:129])
        w[f'w{nm}_B1'] = _to_bf(WT[131:259, 128:129])
        c1x = np.zeros((4, 1), np.float32)
        c1x[0:3, 0] = WT[0:3, 128]
        c1x[3, 0] = bias[128]
        w[f'w{nm}_c1'] = _to_bf(c1x)
    WgT = nl['Wg'].T              # [259, 129]
    w['wg_A'] = _to_bf(WgT[3:131, :])    # [128, 129] (rhs)
    w['wg_B'] = _to_bf(WgT[131:259, :])
    w['wg_c'] = _to_bf(WgT[0:3, :])      # [3, 129]
    WwT = nl['Ww'].T              # [129, 259]
    bw_t = nl['bw'] + nl['Ww'] @ nl['bg']     # fold bg
    # c-chunks of output: {0:3, 3:131, 131:259}
    for tag, sl in (('c', slice(0, 3)), ('A', slice(3, 131)), ('B', slice(131, 259))):
        w[f'ww0_{tag}'] = _to_bf(WwT[0:128, sl])
        top = np.zeros((2, sl.stop - sl.start), np.float32)
        top[0, :] = WwT[128, sl]
        top[1, :] = bw_t[sl]
        w[f'ww1_{tag}'] = _to_bf(top)

    # ---- tower3 ----
    t3 = [blk(x) for x in p['netR_3']]
    (U1, d1, e1, ee1), (U2, d2, e2, ee2), (U3, d3, e3, ee3) = t3
    U1s = e1[:, None] * U1        # [256, 259]
    bt1 = e1 * d1 + ee1
    for m in range(2):
        w[f't3w1_A{m}'] = _to_bf(U1s[:, 3:131].T[:, 128 * m:128 * m + 128])
        w[f't3w1_B{m}'] = _to_bf(U1s[:, 131:259].T[:, 128 * m:128 * m + 128])
        cc = np.zeros((4, 128), np.float32)
        cc[0:3, :] = U1s[:, 0:3].T[:, 128 * m:128 * m + 128]
        cc[3, :] = bt1[128 * m:128 * m + 128]
        w[f't3w1_c{m}'] = _to_bf(cc)
    U2s = e2[:, None] * U2        # [512, 256]
    bt2 = e2 * d2 + ee2
    for m in range(4):
        for kk in range(2):
            w[f't3w2_{kk}{m}'] = _to_bf(U2s.T[128 * kk:128 * kk + 128, 128 * m:128 * m + 128])
        w[f't3w2_b{m}'] = _to_bf(bt2[128 * m:128 * m + 128].reshape(1, 128))
    U3s = e3[:, None] * U3        # [1024, 512]
    bt3 = e3 * d3 + ee3
    for m in range(8):
        for kk in range(4):
            w[f't3w3_{kk}{m}'] = _to_bf(U3s.T[128 * kk:128 * kk + 128, 128 * m:128 * m + 128])
        w[f't3w3_b{m}'] = _to_bf(bt3[128 * m:128 * m + 128].reshape(1, 128))
    g3m, be3m = [np.asarray(a, np.float32) for a in p['max3_bn']]
    for m in range(8):
        w[f'm3s_{m}'] = _to_f32(g3m[128 * m:128 * m + 128].reshape(128, 1))
        w[f'm3b_{m}'] = _to_f32(be3m[128 * m:128 * m + 128].reshape(128, 1))

    # ---- fc ----
    F1, fb1, fg1, fbe1 = blk(p['fc1'])
    F1s = fg1[:, None] * F1       # [256, 1024]
    bf1 = fg1 * fb1 + fbe1
    for m in range(2):
        for kk in range(8):
            w[f'fw1_{kk}{m}'] = _to_bf(F1s.T[128 * kk:128 * kk + 128, 128 * m:128 * m + 128])
        w[f'fw1_b{m}'] = _to_bf(bf1[128 * m:128 * m + 128].reshape(1, 128))
    F2, fb2 = blk(p['fc2'])
    w['fw2_0'] = _to_bf(F2.T[0:128, :])       # [128, 60]
    w['fw2_1'] = _to_bf(F2.T[128:256, :])
    w['fw2_b'] = _to_bf(fb2.reshape(1, 60))

    # ---- consts ----
    w['ident'] = _to_f32(np.eye(128, dtype=np.float32))
    w['pid1'] = np.ascontiguousarray(
        np.broadcast_to((np.arange(N1) + 1).astype(np.float16)[None, :], (128, N1)))
    w['ncol'] = np.ascontiguousarray(np.arange(128, dtype=np.float16).reshape(128, 1))
    return w


def _weight_specs():
    """(name, shape, dtype_str) for every weight tensor, matching _prep_weights."""
    specs = []
    f32, bf, f16 = 'float32', 'bfloat16', 'float16'
    specs += [('t1w1', [128, 128], f32), ('t1s1', [128, 1], f32), ('t1b1', [128, 1], f32),
              ('t1w2', [128, 128], bf), ('t1s2', [128, 1], f32), ('t1b2', [128, 1], f32),
              ('t1w3', [128, 128], bf), ('t1s3', [128, 1], f32), ('t1b3', [128, 1], f32)]
    specs += [('w2fa', [128, 128], bf), ('w2fb', [3, 128], bf), ('w2cn', [3, 128], bf),
              ('b2l1', [128, 1], f32), ('w2l2', [128, 128], bf),
              ('s2l2', [128, 1], f32), ('b2l2', [128, 1], f32),
              ('w2l3a', [128, 128], bf), ('w2l3b', [128, 128], bf),
              ('s2l3a', [128, 1], f32), ('b2l3a', [128, 1], f32),
              ('s2l3b', [128, 1], f32), ('b2l3b', [128, 1], f32)]
    for nm in ('th', 'ph'):
        specs += [(f'w{nm}_A0', [128, 128], bf), (f'w{nm}_B0', [128, 128], bf),
                  (f'w{nm}_c0', [4, 128], bf), (f'w{nm}_A1', [128, 1], bf),
                  (f'w{nm}_B1', [128, 1], bf), (f'w{nm}_c1', [4, 1], bf)]
    specs += [('wg_A', [128, 129], bf), ('wg_B', [128, 129], bf), ('wg_c', [3, 129], bf)]
    for tag, n in (('c', 3), ('A', 128), ('B', 128)):
        specs += [(f'ww0_{tag}', [128, n], bf), (f'ww1_{tag}', [2, n], bf)]
    for m in range(2):
        specs += [(f't3w1_A{m}', [128, 128], bf), (f't3w1_B{m}', [128, 128], bf),
                  (f't3w1_c{m}', [4, 128], bf)]
    for m in range(4):
        specs += [(f't3w2_{kk}{m}', [128, 128], bf) for kk in range(2)]
        specs += [(f't3w2_b{m}', [1, 128], bf)]
    for m in range(8):
        specs += [(f't3w3_{kk}{m}', [128, 128], bf) for kk in range(4)]
        specs += [(f't3w3_b{m}', [1, 128], bf)]
    for m in range(8):
        specs += [(f'm3s_{m}', [128, 1], f32), (f'm3b_{m}', [128, 1], f32)]
    for m in range(2):
        specs += [(f'fw1_{kk}{m}', [128, 128], bf) for kk in range(8)]
        specs += [(f'fw1_b{m}', [1, 128], bf)]
    specs += [('fw2_0', [128, 60], bf), ('fw2_1', [128, 60], bf), ('fw2_b', [1, 60], bf)]
    specs += [('ident', [128, 128], f32), ('pid1', [128, N1], f16), ('ncol', [128, 1], f16)]
    return specs


def build(debug=False):
    import concourse.mybir as mybir
    from concourse.bacc import Bacc
    from concourse.tile import TileContext

    dt = mybir.dt
    AF = mybir.ActivationFunctionType
    AL = mybir.AluOpType

    nc = Bacc(name="attension_point")
    xt4 = nc.dram_tensor("xt4", [SPC, 6, POS], dt.float32, kind="ExternalInput")
    yt4 = nc.dram_tensor("yt4", [SPC, 3, N1], dt.float32, kind="ExternalInput")
    wt = {}
    for name, shape, ds in _weight_specs():
        wt[name] = nc.dram_tensor(name, shape, getattr(dt, ds), kind="ExternalInput")
    out_d = nc.dram_tensor("out", [60, SPC], dt.float32, kind="ExternalOutput")
    dbg = {}
    if debug:
        dbg['feat'] = nc.dram_tensor("dbg_feat", [SPC, 128, N1], dt.float32, kind="ExternalOutput")
        dbg['idx'] = nc.dram_tensor("dbg_idx", [SPC, 128, K], dt.int32, kind="ExternalOutput")
        dbg['out2'] = nc.dram_tensor("dbg_out2", [SPC, 256, 128], dt.float32, kind="ExternalOutput")
        dbg['x3'] = nc.dram_tensor("dbg_x3", [SPC, 259, 128], dt.float32, kind="ExternalOutput")

    with TileContext(nc) as tc:
        import contextlib
        ctx = contextlib.ExitStack()
        with ctx:
            wpool = ctx.enter_context(tc.tile_pool(name="wpool", bufs=1))
            sp = ctx.enter_context(tc.tile_pool(name="sp", bufs=2))          # staging/transients
            bigp = ctx.enter_context(tc.tile_pool(name="bigp", bufs=1))      # big per-sample
            keep = ctx.enter_context(tc.tile_pool(name="keep", bufs=1))      # batch-persistent
            pl1 = ctx.enter_context(tc.tile_pool(name="pl1", bufs=2, space="PSUM"))
            pl2 = ctx.enter_context(tc.tile_pool(name="pl2", bufs=2, space="PSUM"))
            pl3 = ctx.enter_context(tc.tile_pool(name="pl3", bufs=2, space="PSUM"))
            pmisc = ctx.enter_context(tc.tile_pool(name="pmisc", bufs=2, space="PSUM"))
            dpool = ctx.enter_context(tc.tile_pool(name="dpool", bufs=2, space="DRAM"))

            # ---- load weights ----
            W = {}
            for name, shape, ds in _weight_specs():
                t = wpool.tile(shape, getattr(dt, ds), tag=name, name="w_" + name)
                nc.sync.dma_start(t[:], wt[name][:])
                W[name] = t

            # batch-persistent tiles
            x2A = keep.tile([128, 512], dt.bfloat16, tag="x2A")
            x2B = keep.tile([128, 512], dt.bfloat16, tag="x2B")
            x2c = keep.tile([4, 512], dt.bfloat16, tag="x2c")
            nc.vector.memset(x2c[0:4, :], 1.0)
            thb = keep.tile([128, 512], dt.bfloat16, tag="thb")
            thb2 = keep.tile([1, 512], dt.bfloat16, tag="thb2")
            phb = keep.tile([128, 512], dt.bfloat16, tag="phb")
            phb2 = keep.tile([1, 512], dt.bfloat16, tag="phb2")
            gxt = [keep.tile([128, 129], dt.bfloat16, tag=f"gxt{s}", name=f"gxt{s}") for s in range(SPC)]
            ytb = keep.tile([128, 512], dt.bfloat16, tag="ytb")
            ytb2 = keep.tile([2, 512], dt.bfloat16, tag="ytb2")
            nc.vector.memset(ytb2[0:2, :], 1.0)
            ones1 = keep.tile([1, 512], dt.bfloat16, tag="ones1")
            nc.vector.memset(ones1[:], 1.0)
            ones4 = keep.tile([1, SPC], dt.bfloat16, tag="ones4")
            nc.vector.memset(ones4[:], 1.0)

            identb = keep.tile([128, 128], dt.bfloat16, tag="identb")
            nc.vector.tensor_copy(identb[:], W['ident'][:])

            for s in range(SPC):
                # ============ stage A: tower1 + max over k ============
                m1 = bigp.tile([128, 512], dt.float32, tag="m1")
                for r in range(8):
                    x1s = sp.tile([128, 512], dt.float32, tag="x1s")
                    for g in range(4):
                        for sb in range(2):
                            bi = 8 * r + 2 * g + sb
                            nc.sync.dma_start(
                                x1s[32 * g + 6 * sb:32 * g + 6 * sb + 6, :],
                                xt4[s, :, 512 * bi:512 * bi + 512])
                    for g in range(4):
                        ps1 = pl1.tile([128, 512], dt.float32, tag="ps1")
                        nc.tensor.matmul(ps1[:], W['t1w1'][32 * g:32 * g + 12, :],
                                         x1s[32 * g:32 * g + 12, :],
                                         start=True, stop=True, tile_position=(32 * g, 0))
                        h1t = sp.tile([128, 512], dt.bfloat16, tag="h1t")
                        nc.scalar.activation(h1t[:], ps1[:], AF.Relu,
                                             bias=W['t1b1'][:], scale=W['t1s1'][:])
                        ps2 = pl2.tile([128, 512], dt.float32, tag="ps2")
                        nc.tensor.matmul(ps2[:], W['t1w2'][:], h1t[:], start=True, stop=True)
                        h2t = sp.tile([128, 512], dt.bfloat16, tag="h2t")
                        nc.scalar.activation(h2t[:], ps2[:], AF.Relu,
                                             bias=W['t1b2'][:], scale=W['t1s2'][:])
                        for sb in range(2):
                            bi = 8 * r + 2 * g + sb
                            ps3 = pl3.tile([128, 512], dt.float32, tag="ps3")
                            nc.tensor.matmul(ps3[:], W['t1w3'][64 * sb:64 * sb + 64, :],
                                             h2t[64 * sb:64 * sb + 64, :],
                                             start=True, stop=True,
                                             tile_position=(64 * sb, 0))
                            nc.vector.tensor_reduce(
                                m1[:, 8 * bi:8 * bi + 8],
                                ps3[:].rearrange("p (a k) -> p a k", a=8, k=K),
                                axis=mybir.AxisListType.X, op=AL.max)
                # feat = relu(g3*m1 + b3~)  (bf16, kept as lhsT for FG)
                featb = bigp.tile([128, 512], dt.bfloat16, tag="featb")
                nc.scalar.activation(featb[:], m1[:], AF.Relu,
                                     bias=W['t1b3'][:], scale=W['t1s3'][:])
                if debug:
                    featd = sp.tile([128, 512], dt.float32, tag="featd")
                    nc.vector.tensor_copy(featd[:], featb[:])
                    nc.sync.dma_start(dbg['feat'][s], featd[:])

                # ============ stage B: selection ============
                xyz = sp.tile([3, N1], dt.float32, tag="xyz")
                nc.sync.dma_start(xyz[:], yt4[s])
                xyzb = sp.tile([3, N1], dt.bfloat16, tag="xyzb")
                nc.scalar.activation(xyzb[:], xyz[:], AF.Copy)
                # centers^T via PE transpose
                psct = pmisc.tile([128, 128], dt.float32, tag="pm")
                nc.tensor.transpose(psct[0:128, 0:3], xyz[0:3, 0:128], W['ident'][0:3, 0:3])
                cts = sp.tile([128, 3], dt.float32, tag="cts")
                nc.scalar.activation(cts[:], psct[:, 0:3], AF.Copy)
                # d2 exact: sum_c (bcast_c - cT_c)^2
                d2 = sp.tile([128, N1], dt.float32, tag="d2")
                tdiff = sp.tile([128, N1], dt.float32, tag="tdiff")
                tsq = sp.tile([128, N1], dt.float32, tag="tsq")
                for c in range(3):
                    bc = sp.tile([128, N1], dt.float32, tag="bc")
                    nc.sync.dma_start(bc[:], yt4[s, c:c + 1, :].to_broadcast([128, N1]))
                    nc.vector.tensor_scalar(tdiff[:], bc[:], cts[:, c:c + 1], None,
                                            op0=AL.subtract)
                    if c == 0:
                        nc.vector.tensor_mul(d2[:], tdiff[:], tdiff[:])
                    else:
                        nc.vector.tensor_mul(tsq[:], tdiff[:], tdiff[:])
                        nc.vector.tensor_add(d2[:], d2[:], tsq[:])
                # mask / ranks / scatter indices
                maskt = sp.tile([128, N1], dt.float32, tag="maskt")
                nc.vector.tensor_scalar(maskt[:], d2[:], RADIUS2, None, op0=AL.is_le)
                ranks = sp.tile([128, N1], dt.float32, tag="ranks")
                nc.vector.tensor_tensor_scan(ranks[:], maskt[:], maskt[:], 0.0,
                                             op0=AL.add, op1=AL.bypass)
                rm = sp.tile([128, N1], dt.float32, tag="rm")
                nc.vector.scalar_tensor_tensor(rm[:], in0=ranks[:], scalar=1.0,
                                               in1=maskt[:], op0=AL.mult, op1=AL.mult)
                sidx = sp.tile([128, N1], dt.int16, tag="sidx")
                nc.vector.tensor_scalar(sidx[:], rm[:], -1.0, None, op0=AL.add)
                scat = sp.tile([128, K], dt.float16, tag="scat")
                if os.environ.get("STUB_SCATTER"):
                    nc.vector.memset(scat[:], 1.0)
                else:
                    nc.gpsimd.local_scatter(scat[:], W['pid1'][:], sidx[:],
                                            channels=128, num_elems=K, num_idxs=N1)
                m0u = sp.tile([128, K], dt.uint16, tag="m0u")
                nc.vector.tensor_scalar(m0u[:], scat[:], 0.0, None, op0=AL.is_equal)
                sm1 = sp.tile([128, K], dt.float16, tag="sm1")
                nc.vector.tensor_scalar(sm1[:], scat[:], -1.0, None, op0=AL.add)
                idxf = sp.tile([128, K], dt.float16, tag="idxf")
                nc.vector.select(idxf[:], m0u[:], W['ncol'][:].to_broadcast([128, K]), sm1[:])
                idx16 = sp.tile([128, K], dt.int16, tag="idx16")
                nc.vector.tensor_copy(idx16[:], idxf[:])
                if debug:
                    idxo = sp.tile([128, K], dt.int32, tag="idxo")
                    nc.vector.tensor_copy(idxo[:], idx16[:])
                    nc.sync.dma_start(dbg['idx'][s], idxo[:])
                # wrap via DRAM round-trip, replicate x8
                dflat = dpool.tile([128, K], dt.int16, tag="dflat")
                nc.sync.dma_start(dflat[:], idx16[:])
                idxw = sp.tile([128, N1], dt.int16, tag="idxw")
                dview = dflat[:].rearrange("a b -> (a b)").rearrange("(c p) -> p c", p=16)
                for rb in range(8):
                    nc.sync.dma_start(idxw[16 * rb:16 * rb + 16, :], dview)

                # ============ stage C: tower2 on gathered slots ============
                # FG table [128ch, 512pt] fp32
                psfg = pmisc.tile([128, 512], dt.float32, tag="pm")
                nc.tensor.matmul(psfg[:], W['w2fa'][:], featb[:], start=True, stop=False)
                nc.tensor.matmul(psfg[:], W['w2fb'][:], xyzb[:], start=False, stop=True)
                fgt = bigp.tile([128, 512], dt.float32, tag="fgt")
                nc.scalar.activation(fgt[:], psfg[:], AF.Copy)
                # WCneg [128, 128] fp32
                pswc = pmisc.tile([128, 128], dt.float32, tag="pm")
                nc.tensor.matmul(pswc[:], W['w2cn'][:], xyzb[:, 0:128], start=True, stop=True)
                wcn = sp.tile([128, 128], dt.float32, tag="wcn")
                nc.scalar.activation(wcn[:], pswc[:], AF.Copy)
                # gather
                gth = bigp.tile([128, N2 * K // 4], dt.float32, tag="gth")
                h1c = bigp.tile([128, N2 * K // 4], dt.bfloat16, tag="h1c")
                h2c = bigp.tile([128, N2 * K // 4], dt.bfloat16, tag="h2c")
                o2a = bigp.tile([128, 128], dt.float32, tag="o2a")
                o2b = bigp.tile([128, 128], dt.float32, tag="o2b")
                for half in range(4):
                    q = N2 * K // 4   # 2048 slots per quarter
                    if os.environ.get("STUB_GATHER"):
                        nc.vector.tensor_copy(gth[:, 0:512], fgt[:])
                        nc.vector.tensor_copy(gth[:, 512:1024], fgt[:])
                        nc.vector.tensor_copy(gth[:, 1024:1536], fgt[:])
                        nc.vector.tensor_copy(gth[:, 1536:2048], fgt[:])
                    else:
                        nc.gpsimd.ap_gather(gth[:, :, None], fgt[:, :, None],
                                            idxw[:, 128 * half:128 * half + 128],
                                            channels=128, num_elems=N1, d=1, num_idxs=q)
                    # pre-act = gth + wcn[n] + b1~ ; relu -> bf16
                    pre = sp.tile([128, N2 * K // 4], dt.float32, tag="pre")
                    nc.vector.scalar_tensor_tensor(
                        pre[:].rearrange("p (n k) -> p n k", n=32),
                        in0=gth[:].rearrange("p (n k) -> p n k", n=32),
                        scalar=W['b2l1'][:],
                        in1=wcn[:, 32 * half:32 * half + 32, None].to_broadcast([128, 32, K]),
                        op0=AL.add, op1=AL.add)
                    nc.vector.tensor_scalar(h1c[:], pre[:], 0.0, None, op0=AL.max)
                    # L2
                    for t in range(4):
                        psl2 = pl2.tile([128, 512], dt.float32, tag="ps2")
                        nc.tensor.matmul(psl2[:], W['w2l2'][:], h1c[:, 512 * t:512 * t + 512],
                                         start=True, stop=True)
                        nc.scalar.activation(h2c[:, 512 * t:512 * t + 512], psl2[:], AF.Relu,
                                             bias=W['b2l2'][:], scale=W['s2l2'][:])
                    # L3 + reduce
                    for t in range(4):
                        nb = 32 * half + 8 * t
                        psa = pl3.tile([128, 512], dt.float32, tag="ps3")
                        nc.tensor.matmul(psa[:], W['w2l3a'][:], h2c[:, 512 * t:512 * t + 512],
                                         start=True, stop=True)
                        nc.vector.tensor_reduce(
                            o2a[:, nb:nb + 8],
                            psa[:].rearrange("p (a k) -> p a k", a=8, k=K),
                            axis=mybir.AxisListType.X, op=AL.max)
                        psb = pl3.tile([128, 512], dt.float32, tag="ps3")
                        nc.tensor.matmul(psb[:], W['w2l3b'][:], h2c[:, 512 * t:512 * t + 512],
                                         start=True, stop=True)
                        nc.vector.tensor_reduce(
                            o2b[:, nb:nb + 8],
                            psb[:].rearrange("p (a k) -> p a k", a=8, k=K),
                            axis=mybir.AxisListType.X, op=AL.max)
                # x2 assembly
                nc.scalar.activation(x2A[:, 128 * s:128 * s + 128], o2a[:], AF.Relu,
                                     bias=W['b2l3a'][:], scale=W['s2l3a'][:])
                nc.scalar.activation(x2B[:, 128 * s:128 * s + 128], o2b[:], AF.Relu,
                                     bias=W['b2l3b'][:], scale=W['s2l3b'][:])
                nc.vector.tensor_copy(x2c[0:3, 128 * s:128 * s + 128], xyzb[:, 0:128])

            # ============ stage D: attention (batched + per-sample bits) ============
            if debug:
                x2dA = sp.tile([128, 512], dt.float32, tag="x2dA")
                nc.vector.tensor_copy(x2dA[:], x2A[:])
                for s in range(SPC):
                    nc.sync.dma_start(dbg['out2'][s, 0:128, :], x2dA[:, 128 * s:128 * s + 128])
                x2dB = sp.tile([128, 512], dt.float32, tag="x2dB")
                nc.vector.tensor_copy(x2dB[:], x2B[:])
                for s in range(SPC):
                    nc.sync.dma_start(dbg['out2'][s, 128:256, :], x2dB[:, 128 * s:128 * s + 128])
            # th / ph projections [129, 512]
            for nm, main, extra in (('th', thb, thb2), ('ph', phb, phb2)):
                psm = pmisc.tile([128, 512], dt.float32, tag="pm")
                nc.tensor.matmul(psm[:], W[f'w{nm}_A0'][:], x2A[:], start=True, stop=False)
                nc.tensor.matmul(psm[:], W[f'w{nm}_B0'][:], x2B[:], start=False, stop=False)
                nc.tensor.matmul(psm[:], W[f'w{nm}_c0'][:], x2c[:], start=False, stop=True)
                nc.scalar.activation(main[:], psm[:], AF.Copy)
                pse = pmisc.tile([1, 512], dt.float32, tag="pm")
                nc.tensor.matmul(pse[:], W[f'w{nm}_A1'][:], x2A[:], start=True, stop=False)
                nc.tensor.matmul(pse[:], W[f'w{nm}_B1'][:], x2B[:], start=False, stop=False)
                nc.tensor.matmul(pse[:], W[f'w{nm}_c1'][:], x2c[:], start=False, stop=True)
                nc.scalar.activation(extra[:], pse[:], AF.Copy)
            # gxT per sample [128m, 129]
            for s in range(SPC):
                sl = slice(128 * s, 128 * s + 128)
                psg = pmisc.tile([128, 129], dt.float32, tag="pm")
                nc.tensor.matmul(psg[:], x2A[:, sl], W['wg_A'][:], start=True, stop=False)
                nc.tensor.matmul(psg[:], x2B[:, sl], W['wg_B'][:], start=False, stop=False)
                nc.tensor.matmul(psg[:], x2c[0:3, sl], W['wg_c'][:], start=False, stop=True)
                nc.scalar.activation(gxt[s][:], psg[:], AF.Copy)
            # f, softmax, y^T per sample
            for s in range(SPC):
                sl = slice(128 * s, 128 * s + 128)
                psf = pmisc.tile([128, 128], dt.float32, tag="pm")
                nc.tensor.matmul(psf[:], thb[:, sl], phb[:, sl], start=True, stop=False)
                nc.tensor.matmul(psf[:], thb2[:, sl], phb2[:, sl], start=False, stop=True)
                ef = sp.tile([128, 128], dt.float32, tag="ef")
                sums = sp.tile([128, 1], dt.float32, tag="sums")
                nc.scalar.activation(ef[:], psf[:], AF.Exp, accum_out=sums[:])
                rec = sp.tile([128, 1], dt.float32, tag="rec")
                nc.vector.reciprocal(rec[:], sums[:])
                abf = sp.tile([128, 128], dt.bfloat16, tag="abf")
                nc.scalar.activation(abf[:], ef[:], AF.Copy, scale=rec[:])
                psat = pmisc.tile([128, 128], dt.bfloat16, tag="pm")
                nc.tensor.transpose(psat[:], abf[:], identb[:])
                atb = sp.tile([128, 128], dt.bfloat16, tag="atb")
                nc.vector.tensor_copy(atb[:], psat[:])
                psy = pmisc.tile([128, 128], dt.float32, tag="pm")
                nc.tensor.matmul(psy[:], gxt[s][:, 0:128], atb[:], start=True, stop=True)
                nc.scalar.activation(ytb[:, sl], psy[:], AF.Copy)
                psy2 = pmisc.tile([1, 128], dt.float32, tag="pm")
                nc.tensor.matmul(psy2[:], gxt[s][:, 128:129], atb[:], start=True, stop=True)
                nc.scalar.activation(ytb2[0:1, sl], psy2[:], AF.Copy)
            # wy + residual -> x3 chunks
            x3A = keep.tile([128, 512], dt.bfloat16, tag="x3A")
            x3B = keep.tile([128, 512], dt.bfloat16, tag="x3B")
            x3c = keep.tile([4, 512], dt.bfloat16, tag="x3c")
            nc.vector.memset(x3c[0:4, :], 1.0)
            for tag, dst, src, nrow in (('A', x3A, x2A, 128), ('B', x3B, x2B, 128),
                                        ('c', x3c, x2c, 3)):
                psw = pmisc.tile([128, 512], dt.float32, tag="pm")
                nc.tensor.matmul(psw[0:nrow, :], W[f'ww0_{tag}'][:], ytb[:],
                                 start=True, stop=False)
                nc.tensor.matmul(psw[0:nrow, :], W[f'ww1_{tag}'][:], ytb2[:],
                                 start=False, stop=True)
                nc.vector.scalar_tensor_tensor(dst[0:nrow, :], in0=psw[0:nrow, :], scalar=0.0,
                                               in1=src[0:nrow, :], op0=AL.add, op1=AL.add)
            if debug:
                for tag, src, nrow, off in (('A', x3A, 128, 3), ('B', x3B, 128, 131),
                                            ('c', x3c, 3, 0)):
                    xd = sp.tile([128, 512], dt.float32, tag="x3d")
                    nc.vector.tensor_copy(xd[0:nrow, :], src[0:nrow, :])
                    for s in range(SPC):
                        nc.sync.dma_start(dbg['x3'][s, off:off + nrow, :],
                                          xd[0:nrow, 128 * s:128 * s + 128])

            # ============ stage E: tower3 (batched) ============
            h31 = [keep.tile([128, 512], dt.bfloat16, tag=f"h31_{m}", name=f"h31_{m}") for m in range(2)]
            for m in range(2):
                ps = pmisc.tile([128, 512], dt.float32, tag="pm")
                nc.tensor.matmul(ps[:], W[f't3w1_A{m}'][:], x3A[:], start=True, stop=False)
                nc.tensor.matmul(ps[:], W[f't3w1_B{m}'][:], x3B[:], start=False, stop=False)
                nc.tensor.matmul(ps[:], W[f't3w1_c{m}'][:], x3c[:], start=False, stop=True)
                nc.scalar.activation(h31[m][:], ps[:], AF.Relu)
            h32 = [keep.tile([128, 512], dt.bfloat16, tag=f"h32_{m}", name=f"h32_{m}") for m in range(4)]
            for m in range(4):
                ps = pmisc.tile([128, 512], dt.float32, tag="pm")
                nc.tensor.matmul(ps[:], W[f't3w2_0{m}'][:], h31[0][:], start=True, stop=False)
                nc.tensor.matmul(ps[:], W[f't3w2_1{m}'][:], h31[1][:], start=False, stop=False)
                nc.tensor.matmul(ps[:], W[f't3w2_b{m}'][:], ones1[:], start=False, stop=True)
                nc.scalar.activation(h32[m][:], ps[:], AF.Relu)
            x5 = [keep.tile([128, SPC], dt.bfloat16, tag=f"x5_{m}", name=f"x5_{m}") for m in range(8)]
            for m in range(8):
                ps = pmisc.tile([128, 512], dt.float32, tag="pm")
                for kk in range(4):
                    nc.tensor.matmul(ps[:], W[f't3w3_{kk}{m}'][:], h32[kk][:],
                                     start=(kk == 0), stop=False)
                nc.tensor.matmul(ps[:], W[f't3w3_b{m}'][:], ones1[:], start=False, stop=True)
                mx = sp.tile([128, SPC], dt.float32, tag="mx")
                nc.vector.tensor_reduce(mx[:],
                                        ps[:].rearrange("p (s n) -> p s n", s=SPC),
                                        axis=mybir.AxisListType.X, op=AL.max)
                t1 = sp.tile([128, SPC], dt.float32, tag="t1x")
                nc.vector.tensor_scalar(t1[:], mx[:], 0.0, None, op0=AL.max)
                nc.scalar.activation(x5[m][:], t1[:], AF.Relu,
                                     bias=W[f'm3b_{m}'][:], scale=W[f'm3s_{m}'][:])
            # fc1
            x6 = [keep.tile([128, SPC], dt.bfloat16, tag=f"x6_{m}", name=f"x6_{m}") for m in range(2)]
            for m in range(2):
                ps = pmisc.tile([128, SPC], dt.float32, tag="pm")
                for kk in range(8):
                    nc.tensor.matmul(ps[:], W[f'fw1_{kk}{m}'][:], x5[kk][:],
                                     start=(kk == 0), stop=False)
                nc.tensor.matmul(ps[:], W[f'fw1_b{m}'][:], ones4[:], start=False, stop=True)
                nc.scalar.activation(x6[m][:], ps[:], AF.Relu)
            # fc2
            pso = pmisc.tile([60, SPC], dt.float32, tag="pm")
            nc.tensor.matmul(pso[:], W['fw2_0'][:], x6[0][:], start=True, stop=False)
            nc.tensor.matmul(pso[:], W['fw2_1'][:], x6[1][:], start=False, stop=False)
            nc.tensor.matmul(pso[:], W['fw2_b'][:], ones4[:], start=False, stop=True)
            outs = sp.tile([60, SPC], dt.float32, tag="outs")
            nc.scalar.activation(outs[:], pso[:], AF.Copy)
            nc.sync.dma_start(out_d[:], outs[:])
    nc.compile()
    return nc


def _get_built(debug=False):
    global _built
    key = bool(debug)
    if _built is None or _built[0] != key:
        _built = (key, build(debug=debug))
    return _built[1]


def kernel(xt, yt, params, debug=False, return_debug=False):
    from concourse.bass_utils import run_bass_kernel_spmd
    nc = _get_built(debug=debug)
    w = _prep_weights(params)
    xt = np.asarray(xt, np.float32).reshape(B, 6, POS)
    yt = np.asarray(yt, np.float32).reshape(B, 3, N1)
    in_maps = []
    for c in range(NCORE):
        m = dict(w)
        m['xt4'] = np.ascontiguousarray(xt[SPC * c:SPC * c + SPC])
        m['yt4'] = np.ascontiguousarray(yt[SPC * c:SPC * c + SPC])
        in_maps.append(m)
    res = run_bass_kernel_spmd(nc, in_maps, core_ids=list(range(NCORE)))
    out = np.zeros((B, NUM_CLASS), np.float32)
    for c in range(NCORE):
        out[SPC * c:SPC * c + SPC] = res.results[c]['out'].T
    if return_debug:
        return out, res
    return out


# revision 9
# speedup vs baseline: 29.6266x; 29.6266x over previous
"""Trainium2 Bass kernel for nn_Attension_Point (PointNet++-style grouping +
non-local attention + classifier head).

Sharding: pure data parallel — batch 32 split as 4 samples per NeuronCore
across 8 cores; all parameters replicated. Each core runs an identical NEFF
(SPMD) taking [4, ...] inputs and producing [60, 4] logits; the host
reassembles [32, 60].

Self-contained: hardcodes all shapes for the fixed problem size
B=32, D_IN=6, N1=512, K=64, N2=128, NUM_CLASS=60.
"""
import os
import numpy as np
import ml_dtypes

B, D_IN, N1, K, N2, NUM_CLASS = 32, 6, 512, 64, 128, 60
NCORE = 8
SPC = B // NCORE          # samples per core = 4
RADIUS2 = 0.5
POS = N1 * K              # 32768 positions per sample
NBLK = POS // 512         # 64 blocks of 512 positions (8 n-points each)

BF = ml_dtypes.bfloat16

_built = None


def _to_bf(x):
    return np.ascontiguousarray(np.asarray(x, np.float32).astype(BF))


def _to_f32(x):
    return np.ascontiguousarray(np.asarray(x, np.float32))


def _prep_weights(params):
    """Host-side parameter preparation into device layouts."""
    p = params
    w = {}

    def blk(b):
        return tuple(np.asarray(a, np.float32) for a in b)

    # ---- tower1 ----
    (W1, b1, g1, be1), (W2, b2, g2, be2), (W3, b3, g3, be3) = [blk(x) for x in p['netR_1']]
    t1w1 = np.zeros((128, 128), np.float32)
    for g in range(4):
        t1w1[32 * g:32 * g + 6, 0:64] = W1.T
        t1w1[32 * g + 6:32 * g + 12, 64:128] = W1.T
    w['t1w1'] = _to_bf(t1w1)
    w['t1s1'] = _to_f32(np.tile(g1, 2).reshape(128, 1))
    w['t1b1'] = _to_f32(np.tile(g1 * b1 + be1, 2).reshape(128, 1))
    t1w2 = np.zeros((128, 128), np.float32)
    t1w2[0:64, 0:64] = W2.T
    t1w2[64:128, 64:128] = W2.T
    w['t1w2'] = _to_bf(t1w2)
    w['t1s2'] = _to_f32(np.tile(g2, 2).reshape(128, 1))
    w['t1b2'] = _to_f32(np.tile(g2 * b2 + be2, 2).reshape(128, 1))
    t1w3 = np.zeros((128, 128), np.float32)
    t1w3[0:64, :] = W3.T
    t1w3[64:128, :] = W3.T
    w['t1w3'] = _to_bf(t1w3)
    w['t1s3'] = _to_f32(g3.reshape(128, 1))
    w['t1b3'] = _to_f32((g3 * b3 + be3).reshape(128, 1))

    # ---- tower2 (netR_2): input ch order [xyz(3); feat(128)] ----
    (V1, c1, h1, he1), (V2, c2, h2, he2), (V3, c3, h3, he3) = [blk(x) for x in p['netR_2']]
    V1s = h1[:, None] * V1            # fold bn scale into weights
    w['w2fa'] = _to_bf(V1s[:, 3:].T)          # [128, 128] feat part (lhsT)
    w['w2fb'] = _to_bf(V1s[:, :3].T)          # [3, 128] xyz part
    w['w2cn'] = _to_bf(-V1s[:, :3].T)         # [3, 128] for WCneg
    w['b2l1'] = _to_f32((h1 * c1 + he1).reshape(128, 1))
    w['w2l2'] = _to_bf(V2.T)                   # [128, 128]
    w['s2l2'] = _to_f32(h2.reshape(128, 1))
    w['b2l2'] = _to_f32((h2 * c2 + he2).reshape(128, 1))
    w['w2l3a'] = _to_bf(V3[0:128, :].T)        # [128, 128]
    w['w2l3b'] = _to_bf(V3[128:256, :].T)
    w['s2l3a'] = _to_f32(h3[0:128].reshape(128, 1))
    w['b2l3a'] = _to_f32((h3 * c3 + he3)[0:128].reshape(128, 1))
    w['s2l3b'] = _to_f32(h3[128:256].reshape(128, 1))
    w['b2l3b'] = _to_f32((h3 * c3 + he3)[128:256].reshape(128, 1))

    # ---- attention ----
    nl = {k: np.asarray(v, np.float32) for k, v in p['nl'].items()}
    # x chunks along contraction c: A = ch 3:131, B = ch 131:259, c = ch 0:3 (+ones)
    for nm in ('th', 'ph'):
        W = nl['W' + nm]          # [129, 259]
        bias = nl['b' + nm]       # [129]
        WT = W.T                  # [259, 129]
        w[f'w{nm}_A0'] = _to_bf(WT[3:131, 0:128])
        w[f'w{nm}_B0'] = _to_bf(WT[131:259, 0:128])
        c0 = np.zeros((4, 128), np.float32)
        c0[0:3, :] = WT[0:3, 0:128]
        c0[3, :] = bias[0:128]
        w[f'w{nm}_c0'] = _to_bf(c0)
        w[f'w{nm}_A1'] = _to_bf(WT[3:131, 128:129])
        w[f'w{nm}_B1'] = _to_bf(WT[131:259, 128:129])
        c1x = np.zeros((4, 1), np.float32)
        c1x[0:3, 0] = WT[0:3, 128]
        c1x[3, 0] = bias[128]
        w[f'w{nm}_c1'] = _to_bf(c1x)
    WgT = nl['Wg'].T              # [259, 129]
    w['wg_A'] = _to_bf(WgT[3:131, :])    # [128, 129] (rhs)
    w['wg_B'] = _to_bf(WgT[131:259, :])
    w['wg_c'] = _to_bf(WgT[0:3, :])      # [3, 129]
    WwT = nl['Ww'].T              # [129, 259]
    bw_t = nl['bw'] + nl['Ww'] @ nl['bg']     # fold bg
    # c-chunks of output: {0:3, 3:131, 131:259}
    for tag, sl in (('c', slice(0, 3)), ('A', slice(3, 131)), ('B', slice(131, 259))):
        w[f'ww0_{tag}'] = _to_bf(WwT[0:128, sl])
        top = np.zeros((2, sl.stop - sl.start), np.float32)
        top[0, :] = WwT[128, sl]
        top[1, :] = bw_t[sl]
        w[f'ww1_{tag}'] = _to_bf(top)

    # ---- tower3 ----
    t3 = [blk(x) for x in p['netR_3']]
    (U1, d1, e1, ee1), (U2, d2, e2, ee2), (U3, d3, e3, ee3) = t3
    U1s = e1[:, None] * U1        # [256, 259]
    bt1 = e1 * d1 + ee1
    for m in range(2):
        w[f't3w1_A{m}'] = _to_bf(U1s[:, 3:131].T[:, 128 * m:128 * m + 128])
        w[f't3w1_B{m}'] = _to_bf(U1s[:, 131:259].T[:, 128 * m:128 * m + 128])
        cc = np.zeros((4, 128), np.float32)
        cc[0:3, :] = U1s[:, 0:3].T[:, 128 * m:128 * m + 128]
        cc[3, :] = bt1[128 * m:128 * m + 128]
        w[f't3w1_c{m}'] = _to_bf(cc)
    U2s = e2[:, None] * U2        # [512, 256]
    bt2 = e2 * d2 + ee2
    for m in range(4):
        for kk in range(2):
            w[f't3w2_{kk}{m}'] = _to_bf(U2s.T[128 * kk:128 * kk + 128, 128 * m:128 * m + 128])
        w[f't3w2_b{m}'] = _to_bf(bt2[128 * m:128 * m + 128].reshape(1, 128))
    U3s = e3[:, None] * U3        # [1024, 512]
    bt3 = e3 * d3 + ee3
    for m in range(8):
        for kk in range(4):
            w[f't3w3_{kk}{m}'] = _to_bf(U3s.T[128 * kk:128 * kk + 128, 128 * m:128 * m + 128])
        w[f't3w3_b{m}'] = _to_bf(bt3[128 * m:128 * m + 128].reshape(1, 128))
    g3m, be3m = [np.asarray(a, np.float32) for a in p['max3_bn']]
    for m in range(8):
        w[f'm3s_{m}'] = _to_f32(g3m[128 * m:128 * m + 128].reshape(128, 1))
        w[f'm3b_{m}'] = _to_f32(be3m[128 * m:128 * m + 128].reshape(128, 1))

    # ---- fc ----
    F1, fb1, fg1, fbe1 = blk(p['fc1'])
    F1s = fg1[:, None] * F1       # [256, 1024]
    bf1 = fg1 * fb1 + fbe1
    for m in range(2):
        for kk in range(8):
            w[f'fw1_{kk}{m}'] = _to_bf(F1s.T[128 * kk:128 * kk + 128, 128 * m:128 * m + 128])
        w[f'fw1_b{m}'] = _to_bf(bf1[128 * m:128 * m + 128].reshape(1, 128))
    F2, fb2 = blk(p['fc2'])
    w['fw2_0'] = _to_bf(F2.T[0:128, :])       # [128, 60]
    w['fw2_1'] = _to_bf(F2.T[128:256, :])
    w['fw2_b'] = _to_bf(fb2.reshape(1, 60))

    # ---- consts ----
    w['ident'] = _to_f32(np.eye(128, dtype=np.float32))
    w['pid1'] = np.ascontiguousarray(
        np.broadcast_to((np.arange(N1) + 1).astype(np.float16)[None, :], (128, N1)))
    w['ncol'] = np.ascontiguousarray(np.arange(128, dtype=np.float16).reshape(128, 1))
    return w


def _weight_specs():
    """(name, shape, dtype_str) for every weight tensor, matching _prep_weights."""
    specs = []
    f32, bf, f16 = 'float32', 'bfloat16', 'float16'
    specs += [('t1w1', [128, 128], bf), ('t1s1', [128, 1], f32), ('t1b1', [128, 1], f32),
              ('t1w2', [128, 128], bf), ('t1s2', [128, 1], f32), ('t1b2', [128, 1], f32),
              ('t1w3', [128, 128], bf), ('t1s3', [128, 1], f32), ('t1b3', [128, 1], f32)]
    specs += [('w2fa', [128, 128], bf), ('w2fb', [3, 128], bf), ('w2cn', [3, 128], bf),
              ('b2l1', [128, 1], f32), ('w2l2', [128, 128], bf),
              ('s2l2', [128, 1], f32), ('b2l2', [128, 1], f32),
              ('w2l3a', [128, 128], bf), ('w2l3b', [128, 128], bf),
              ('s2l3a', [128, 1], f32), ('b2l3a', [128, 1], f32),
              ('s2l3b', [128, 1], f32), ('b2l3b', [128, 1], f32)]
    for nm in ('th', 'ph'):
        specs += [(f'w{nm}_A0', [128, 128], bf), (f'w{nm}_B0', [128, 128], bf),
                  (f'w{nm}_c0', [4, 128], bf), (f'w{nm}_A1', [128, 1], bf),
                  (f'w{nm}_B1', [128, 1], bf), (f'w{nm}_c1', [4, 1], bf)]
    specs += [('wg_A', [128, 129], bf), ('wg_B', [128, 129], bf), ('wg_c', [3, 129], bf)]
    for tag, n in (('c', 3), ('A', 128), ('B', 128)):
        specs += [(f'ww0_{tag}', [128, n], bf), (f'ww1_{tag}', [2, n], bf)]
    for m in range(2):
        specs += [(f't3w1_A{m}', [128, 128], bf), (f't3w1_B{m}', [128, 128], bf),
                  (f't3w1_c{m}', [4, 128], bf)]
    for m in range(4):
        specs += [(f't3w2_{kk}{m}', [128, 128], bf) for kk in range(2)]
        specs += [(f't3w2_b{m}', [1, 128], bf)]
    for m in range(8):
        specs += [(f't3w3_{kk}{m}', [128, 128], bf) for kk in range(4)]
        specs += [(f't3w3_b{m}', [1, 128], bf)]
    for m in range(8):
        specs += [(f'm3s_{m}', [128, 1], f32), (f'm3b_{m}', [128, 1], f32)]
    for m in range(2):
        specs += [(f'fw1_{kk}{m}', [128, 128], bf) for kk in range(8)]
        specs += [(f'fw1_b{m}', [1, 128], bf)]
    specs += [('fw2_0', [128, 60], bf), ('fw2_1', [128, 60], bf), ('fw2_b', [1, 60], bf)]
    specs += [('ident', [128, 128], f32), ('pid1', [128, N1], f16), ('ncol', [128, 1], f16)]
    return specs


def build(debug=False):
    import concourse.mybir as mybir
    from concourse.bacc import Bacc
    from concourse.tile import TileContext

    dt = mybir.dt
    AF = mybir.ActivationFunctionType
    AL = mybir.AluOpType

    nc = Bacc(name="attension_point")
    xt4 = nc.dram_tensor("xt4", [SPC, 8, 128, 512], dt.bfloat16, kind="ExternalInput")
    yt4 = nc.dram_tensor("yt4", [SPC, 3, N1], dt.float32, kind="ExternalInput")
    wt = {}
    for name, shape, ds in _weight_specs():
        wt[name] = nc.dram_tensor(name, shape, getattr(dt, ds), kind="ExternalInput")
    out_d = nc.dram_tensor("out", [60, SPC], dt.float32, kind="ExternalOutput")
    dbg = {}
    if debug:
        dbg['feat'] = nc.dram_tensor("dbg_feat", [SPC, 128, N1], dt.float32, kind="ExternalOutput")
        dbg['idx'] = nc.dram_tensor("dbg_idx", [SPC, 128, K], dt.int32, kind="ExternalOutput")
        dbg['out2'] = nc.dram_tensor("dbg_out2", [SPC, 256, 128], dt.float32, kind="ExternalOutput")
        dbg['x3'] = nc.dram_tensor("dbg_x3", [SPC, 259, 128], dt.float32, kind="ExternalOutput")

    with TileContext(nc) as tc:
        import contextlib
        ctx = contextlib.ExitStack()
        with ctx:
            wpool = ctx.enter_context(tc.tile_pool(name="wpool", bufs=1))
            sp = ctx.enter_context(tc.tile_pool(name="sp", bufs=2))          # staging/transients
            bigp = ctx.enter_context(tc.tile_pool(name="bigp", bufs=2))      # big per-sample
            keep = ctx.enter_context(tc.tile_pool(name="keep", bufs=1))      # batch-persistent
            pl1 = ctx.enter_context(tc.tile_pool(name="pl1", bufs=2, space="PSUM"))
            pl2 = ctx.enter_context(tc.tile_pool(name="pl2", bufs=2, space="PSUM"))
            pl3 = ctx.enter_context(tc.tile_pool(name="pl3", bufs=2, space="PSUM"))
            pmisc = ctx.enter_context(tc.tile_pool(name="pmisc", bufs=2, space="PSUM"))
            dpool = ctx.enter_context(tc.tile_pool(name="dpool", bufs=2, space="DRAM"))

            # ---- load weights ----
            W = {}
            for name, shape, ds in _weight_specs():
                t = wpool.tile(shape, getattr(dt, ds), tag=name, name="w_" + name)
                nc.sync.dma_start(t[:], wt[name][:])
                W[name] = t

            # batch-persistent tiles
            x2A = keep.tile([128, 512], dt.bfloat16, tag="x2A")
            x2B = keep.tile([128, 512], dt.bfloat16, tag="x2B")
            x2c = keep.tile([4, 512], dt.bfloat16, tag="x2c")
            nc.vector.memset(x2c[0:4, :], 1.0)
            thb = keep.tile([128, 512], dt.bfloat16, tag="thb")
            thb2 = keep.tile([1, 512], dt.bfloat16, tag="thb2")
            phb = keep.tile([128, 512], dt.bfloat16, tag="phb")
            phb2 = keep.tile([1, 512], dt.bfloat16, tag="phb2")
            gxt = [keep.tile([128, 129], dt.bfloat16, tag=f"gxt{s}", name=f"gxt{s}") for s in range(SPC)]
            ytb = keep.tile([128, 512], dt.bfloat16, tag="ytb")
            ytb2 = keep.tile([2, 512], dt.bfloat16, tag="ytb2")
            nc.vector.memset(ytb2[0:2, :], 1.0)
            ones1 = keep.tile([1, 512], dt.bfloat16, tag="ones1")
            nc.vector.memset(ones1[:], 1.0)
            ones4 = keep.tile([1, SPC], dt.bfloat16, tag="ones4")
            nc.vector.memset(ones4[:], 1.0)

            identb = keep.tile([128, 128], dt.bfloat16, tag="identb")
            nc.vector.tensor_copy(identb[:], W['ident'][:])

            for s in range(SPC):
                # ============ stage A: tower1 + max over k ============
                m1 = bigp.tile([128, 512], dt.float32, tag="m1")
                for r in range(8):
                    x1s = sp.tile([128, 512], dt.bfloat16, tag="x1s")
                    nc.sync.dma_start(x1s[:], xt4[s, r])
                    for g in range(4):
                        ps1 = pl1.tile([128, 512], dt.float32, tag="ps1")
                        nc.tensor.matmul(ps1[:], W['t1w1'][32 * g:32 * g + 12, :],
                                         x1s[32 * g:32 * g + 12, :],
                                         start=True, stop=True, tile_position=(32 * g, 0))
                        h1t = sp.tile([128, 512], dt.bfloat16, tag="h1t")
                        nc.scalar.activation(h1t[:], ps1[:], AF.Relu,
                                             bias=W['t1b1'][:], scale=W['t1s1'][:])
                        ps2 = pl2.tile([128, 512], dt.float32, tag="ps2")
                        nc.tensor.matmul(ps2[:], W['t1w2'][:], h1t[:], start=True, stop=True)
                        h2t = sp.tile([128, 512], dt.bfloat16, tag="h2t")
                        nc.scalar.activation(h2t[:], ps2[:], AF.Relu,
                                             bias=W['t1b2'][:], scale=W['t1s2'][:])
                        for sb in range(2):
                            bi = 8 * r + 2 * g + sb
                            ps3 = pl3.tile([128, 512], dt.float32, tag="ps3")
                            nc.tensor.matmul(ps3[:], W['t1w3'][64 * sb:64 * sb + 64, :],
                                             h2t[64 * sb:64 * sb + 64, :],
                                             start=True, stop=True,
                                             tile_position=(64 * sb, 0))
                            nc.vector.tensor_reduce(
                                m1[:, 8 * bi:8 * bi + 8],
                                ps3[:].rearrange("p (a k) -> p a k", a=8, k=K),
                                axis=mybir.AxisListType.X, op=AL.max)
                # feat = relu(g3*m1 + b3~)  (bf16, kept as lhsT for FG)
                featb = bigp.tile([128, 512], dt.bfloat16, tag="featb")
                nc.scalar.activation(featb[:], m1[:], AF.Relu,
                                     bias=W['t1b3'][:], scale=W['t1s3'][:])
                if debug:
                    featd = sp.tile([128, 512], dt.float32, tag="featd")
                    nc.vector.tensor_copy(featd[:], featb[:])
                    nc.sync.dma_start(dbg['feat'][s], featd[:])

                # ============ stage B: selection ============
                xyz = sp.tile([3, N1], dt.float32, tag="xyz")
                nc.sync.dma_start(xyz[:], yt4[s])
                xyzb = sp.tile([3, N1], dt.bfloat16, tag="xyzb")
                nc.scalar.activation(xyzb[:], xyz[:], AF.Copy)
                # centers^T via PE transpose
                psct = pmisc.tile([128, 128], dt.float32, tag="pm")
                nc.tensor.transpose(psct[0:128, 0:3], xyz[0:3, 0:128], W['ident'][0:3, 0:3])
                cts = sp.tile([128, 3], dt.float32, tag="cts")
                nc.scalar.activation(cts[:], psct[:, 0:3], AF.Copy)
                # d2 exact: sum_c (bcast_c - cT_c)^2
                d2 = sp.tile([128, N1], dt.float32, tag="d2")
                tdiff = sp.tile([128, N1], dt.float32, tag="tdiff")
                tsq = sp.tile([128, N1], dt.float32, tag="tsq")
                for c in range(3):
                    bc = sp.tile([128, N1], dt.float32, tag="bc")
                    nc.sync.dma_start(bc[:], yt4[s, c:c + 1, :].to_broadcast([128, N1]))
                    nc.vector.tensor_scalar(tdiff[:], bc[:], cts[:, c:c + 1], None,
                                            op0=AL.subtract)
                    if c == 0:
                        nc.vector.tensor_mul(d2[:], tdiff[:], tdiff[:])
                    else:
                        nc.vector.tensor_mul(tsq[:], tdiff[:], tdiff[:])
                        nc.vector.tensor_add(d2[:], d2[:], tsq[:])
                # mask / ranks / scatter indices
                maskt = sp.tile([128, N1], dt.float32, tag="maskt")
                nc.vector.tensor_scalar(maskt[:], d2[:], RADIUS2, None, op0=AL.is_le)
                ranks = sp.tile([128, N1], dt.float32, tag="ranks")
                nc.vector.tensor_tensor_scan(ranks[:], maskt[:], maskt[:], 0.0,
                                             op0=AL.add, op1=AL.bypass)
                rm = sp.tile([128, N1], dt.float32, tag="rm")
                nc.vector.scalar_tensor_tensor(rm[:], in0=ranks[:], scalar=1.0,
                                               in1=maskt[:], op0=AL.mult, op1=AL.mult)
                sidx = sp.tile([128, N1], dt.int16, tag="sidx")
                nc.vector.tensor_scalar(sidx[:], rm[:], -1.0, None, op0=AL.add)
                scat = sp.tile([128, K], dt.float16, tag="scat")
                if os.environ.get("STUB_SCATTER"):
                    nc.vector.memset(scat[:], 1.0)
                else:
                    nc.gpsimd.local_scatter(scat[:], W['pid1'][:], sidx[:],
                                            channels=128, num_elems=K, num_idxs=N1)
                m0u = sp.tile([128, K], dt.uint16, tag="m0u")
                nc.vector.tensor_scalar(m0u[:], scat[:], 0.0, None, op0=AL.is_equal)
                sm1 = sp.tile([128, K], dt.float16, tag="sm1")
                nc.vector.tensor_scalar(sm1[:], scat[:], -1.0, None, op0=AL.add)
                idxf = sp.tile([128, K], dt.float16, tag="idxf")
                nc.vector.select(idxf[:], m0u[:], W['ncol'][:].to_broadcast([128, K]), sm1[:])
                idx16 = sp.tile([128, K], dt.int16, tag="idx16")
                nc.vector.tensor_copy(idx16[:], idxf[:])
                if debug:
                    idxo = sp.tile([128, K], dt.int32, tag="idxo")
                    nc.vector.tensor_copy(idxo[:], idx16[:])
                    nc.sync.dma_start(dbg['idx'][s], idxo[:])
                # wrap via DRAM round-trip, replicate x8
                dflat = dpool.tile([128, K], dt.int16, tag="dflat")
                nc.sync.dma_start(dflat[:], idx16[:])
                idxw = sp.tile([128, N1], dt.int16, tag="idxw")
                dview = dflat[:].rearrange("a b -> (a b)").rearrange("(c p) -> p c", p=16)
                for rb in range(8):
                    nc.sync.dma_start(idxw[16 * rb:16 * rb + 16, :], dview)

                # ============ stage C: tower2 on gathered slots ============
                # FG table [128ch, 512pt] fp32
                psfg = pmisc.tile([128, 512], dt.float32, tag="pm")
                nc.tensor.matmul(psfg[:], W['w2fa'][:], featb[:], start=True, stop=False)
                nc.tensor.matmul(psfg[:], W['w2fb'][:], xyzb[:], start=False, stop=True)
                fgt = bigp.tile([128, 512], dt.float32, tag="fgt")
                nc.scalar.activation(fgt[:], psfg[:], AF.Copy)
                # WCneg [128, 128] fp32
                pswc = pmisc.tile([128, 128], dt.float32, tag="pm")
                nc.tensor.matmul(pswc[:], W['w2cn'][:], xyzb[:, 0:128], start=True, stop=True)
                wcn = sp.tile([128, 128], dt.float32, tag="wcn")
                nc.scalar.activation(wcn[:], pswc[:], AF.Copy)
                # gather
                gth = bigp.tile([128, N2 * K // 4], dt.float32, tag="gth")
                h1c = bigp.tile([128, N2 * K // 4], dt.bfloat16, tag="h1c")
                h2c = bigp.tile([128, N2 * K // 4], dt.bfloat16, tag="h2c")
                o2a = bigp.tile([128, 128], dt.float32, tag="o2a")
                o2b = bigp.tile([128, 128], dt.float32, tag="o2b")
                for half in range(4):
                    q = N2 * K // 4   # 2048 slots per quarter
                    if os.environ.get("STUB_GATHER"):
                        nc.vector.tensor_copy(gth[:, 0:512], fgt[:])
                        nc.vector.tensor_copy(gth[:, 512:1024], fgt[:])
                        nc.vector.tensor_copy(gth[:, 1024:1536], fgt[:])
                        nc.vector.tensor_copy(gth[:, 1536:2048], fgt[:])
                    else:
                        nc.gpsimd.ap_gather(gth[:, :, None], fgt[:, :, None],
                                            idxw[:, 128 * half:128 * half + 128],
                                            channels=128, num_elems=N1, d=1, num_idxs=q)
                    # pre-act = gth + wcn[n] + b1~ ; relu -> bf16
                    pre = sp.tile([128, N2 * K // 4], dt.float32, tag="pre")
                    nc.vector.scalar_tensor_tensor(
                        pre[:].rearrange("p (n k) -> p n k", n=32),
                        in0=gth[:].rearrange("p (n k) -> p n k", n=32),
                        scalar=W['b2l1'][:],
                        in1=wcn[:, 32 * half:32 * half + 32, None].to_broadcast([128, 32, K]),
                        op0=AL.add, op1=AL.add)
                    nc.scalar.activation(h1c[:], pre[:], AF.Relu)
                    # L2
                    for t in range(4):
                        psl2 = pl2.tile([128, 512], dt.float32, tag="ps2")
                        nc.tensor.matmul(psl2[:], W['w2l2'][:], h1c[:, 512 * t:512 * t + 512],
                                         start=True, stop=True)
                        nc.scalar.activation(h2c[:, 512 * t:512 * t + 512], psl2[:], AF.Relu,
                                             bias=W['b2l2'][:], scale=W['s2l2'][:])
                    # L3 + reduce
                    for t in range(4):
                        nb = 32 * half + 8 * t
                        psa = pl3.tile([128, 512], dt.float32, tag="ps3")
                        nc.tensor.matmul(psa[:], W['w2l3a'][:], h2c[:, 512 * t:512 * t + 512],
                                         start=True, stop=True)
                        nc.vector.tensor_reduce(
                            o2a[:, nb:nb + 8],
                            psa[:].rearrange("p (a k) -> p a k", a=8, k=K),
                            axis=mybir.AxisListType.X, op=AL.max)
                        psb = pl3.tile([128, 512], dt.float32, tag="ps3")
                        nc.tensor.matmul(psb[:], W['w2l3b'][:], h2c[:, 512 * t:512 * t + 512],
                                         start=True, stop=True)
                        nc.vector.tensor_reduce(
                            o2b[:, nb:nb + 8],
                            psb[:].rearrange("p (a k) -> p a k", a=8, k=K),
                            axis=mybir.AxisListType.X, op=AL.max)
                # x2 assembly
                nc.scalar.activation(x2A[:, 128 * s:128 * s + 128], o2a[:], AF.Relu,
                                     bias=W['b2l3a'][:], scale=W['s2l3a'][:])
                nc.scalar.activation(x2B[:, 128 * s:128 * s + 128], o2b[:], AF.Relu,
                                     bias=W['b2l3b'][:], scale=W['s2l3b'][:])
                nc.vector.tensor_copy(x2c[0:3, 128 * s:128 * s + 128], xyzb[:, 0:128])

            # ============ stage D: attention (batched + per-sample bits) ============
            if debug:
                x2dA = sp.tile([128, 512], dt.float32, tag="x2dA")
                nc.vector.tensor_copy(x2dA[:], x2A[:])
                for s in range(SPC):
                    nc.sync.dma_start(dbg['out2'][s, 0:128, :], x2dA[:, 128 * s:128 * s + 128])
                x2dB = sp.tile([128, 512], dt.float32, tag="x2dB")
                nc.vector.tensor_copy(x2dB[:], x2B[:])
                for s in range(SPC):
                    nc.sync.dma_start(dbg['out2'][s, 128:256, :], x2dB[:, 128 * s:128 * s + 128])
            # th / ph projections [129, 512]
            for nm, main, extra in (('th', thb, thb2), ('ph', phb, phb2)):
                psm = pmisc.tile([128, 512], dt.float32, tag="pm")
                nc.tensor.matmul(psm[:], W[f'w{nm}_A0'][:], x2A[:], start=True, stop=False)
                nc.tensor.matmul(psm[:], W[f'w{nm}_B0'][:], x2B[:], start=False, stop=False)
                nc.tensor.matmul(psm[:], W[f'w{nm}_c0'][:], x2c[:], start=False, stop=True)
                nc.scalar.activation(main[:], psm[:], AF.Copy)
                pse = pmisc.tile([1, 512], dt.float32, tag="pm")
                nc.tensor.matmul(pse[:], W[f'w{nm}_A1'][:], x2A[:], start=True, stop=False)
                nc.tensor.matmul(pse[:], W[f'w{nm}_B1'][:], x2B[:], start=False, stop=False)
                nc.tensor.matmul(pse[:], W[f'w{nm}_c1'][:], x2c[:], start=False, stop=True)
                nc.scalar.activation(extra[:], pse[:], AF.Copy)
            # gxT per sample [128m, 129]
            for s in range(SPC):
                sl = slice(128 * s, 128 * s + 128)
                psg = pmisc.tile([128, 129], dt.float32, tag="pm")
                nc.tensor.matmul(psg[:], x2A[:, sl], W['wg_A'][:], start=True, stop=False)
                nc.tensor.matmul(psg[:], x2B[:, sl], W['wg_B'][:], start=False, stop=False)
                nc.tensor.matmul(psg[:], x2c[0:3, sl], W['wg_c'][:], start=False, stop=True)
                nc.scalar.activation(gxt[s][:], psg[:], AF.Copy)
            # f, softmax, y^T per sample
            for s in range(SPC):
                sl = slice(128 * s, 128 * s + 128)
                psf = pmisc.tile([128, 128], dt.float32, tag="pm")
                nc.tensor.matmul(psf[:], thb[:, sl], phb[:, sl], start=True, stop=False)
                nc.tensor.matmul(psf[:], thb2[:, sl], phb2[:, sl], start=False, stop=True)
                ef = sp.tile([128, 128], dt.float32, tag="ef")
                sums = sp.tile([128, 1], dt.float32, tag="sums")
                nc.scalar.activation(ef[:], psf[:], AF.Exp, accum_out=sums[:])
                rec = sp.tile([128, 1], dt.float32, tag="rec")
                nc.vector.reciprocal(rec[:], sums[:])
                abf = sp.tile([128, 128], dt.bfloat16, tag="abf")
                nc.scalar.activation(abf[:], ef[:], AF.Copy, scale=rec[:])
                psat = pmisc.tile([128, 128], dt.bfloat16, tag="pm")
                nc.tensor.transpose(psat[:], abf[:], identb[:])
                atb = sp.tile([128, 128], dt.bfloat16, tag="atb")
                nc.vector.tensor_copy(atb[:], psat[:])
                psy = pmisc.tile([128, 128], dt.float32, tag="pm")
                nc.tensor.matmul(psy[:], gxt[s][:, 0:128], atb[:], start=True, stop=True)
                nc.scalar.activation(ytb[:, sl], psy[:], AF.Copy)
                psy2 = pmisc.tile([1, 128], dt.float32, tag="pm")
                nc.tensor.matmul(psy2[:], gxt[s][:, 128:129], atb[:], start=True, stop=True)
                nc.scalar.activation(ytb2[0:1, sl], psy2[:], AF.Copy)
            # wy + residual -> x3 chunks
            x3A = keep.tile([128, 512], dt.bfloat16, tag="x3A")
            x3B = keep.tile([128, 512], dt.bfloat16, tag="x3B")
            x3c = keep.tile([4, 512], dt.bfloat16, tag="x3c")
            nc.vector.memset(x3c[0:4, :], 1.0)
            for tag, dst, src, nrow in (('A', x3A, x2A, 128), ('B', x3B, x2B, 128),
                                        ('c', x3c, x2c, 3)):
                psw = pmisc.tile([128, 512], dt.float32, tag="pm")
                nc.tensor.matmul(psw[0:nrow, :], W[f'ww0_{tag}'][:], ytb[:],
                                 start=True, stop=False)
                nc.tensor.matmul(psw[0:nrow, :], W[f'ww1_{tag}'][:], ytb2[:],
                                 start=False, stop=True)
                nc.vector.scalar_tensor_tensor(dst[0:nrow, :], in0=psw[0:nrow, :], scalar=0.0,
                                               in1=src[0:nrow, :], op0=AL.add, op1=AL.add)
            if debug:
                for tag, src, nrow, off in (('A', x3A, 128, 3), ('B', x3B, 128, 131),
                                            ('c', x3c, 3, 0)):
                    xd = sp.tile([128, 512], dt.float32, tag="x3d")
                    nc.vector.tensor_copy(xd[0:nrow, :], src[0:nrow, :])
                    for s in range(SPC):
                        nc.sync.dma_start(dbg['x3'][s, off:off + nrow, :],
                                          xd[0:nrow, 128 * s:128 * s + 128])

            # ============ stage E: tower3 (batched) ============
            h31 = [keep.tile([128, 512], dt.bfloat16, tag=f"h31_{m}", name=f"h31_{m}") for m in range(2)]
            for m in range(2):
                ps = pmisc.tile([128, 512], dt.float32, tag="pm")
                nc.tensor.matmul(ps[:], W[f't3w1_A{m}'][:], x3A[:], start=True, stop=False)
                nc.tensor.matmul(ps[:], W[f't3w1_B{m}'][:], x3B[:], start=False, stop=False)
                nc.tensor.matmul(ps[:], W[f't3w1_c{m}'][:], x3c[:], start=False, stop=True)
                nc.scalar.activation(h31[m][:], ps[:], AF.Relu)
            h32 = [keep.tile([128, 512], dt.bfloat16, tag=f"h32_{m}", name=f"h32_{m}") for m in range(4)]
            for m in range(4):
                ps = pmisc.tile([128, 512], dt.float32, tag="pm")
                nc.tensor.matmul(ps[:], W[f't3w2_0{m}'][:], h31[0][:], start=True, stop=False)
                nc.tensor.matmul(ps[:], W[f't3w2_1{m}'][:], h31[1][:], start=False, stop=False)
                nc.tensor.matmul(ps[:], W[f't3w2_b{m}'][:], ones1[:], start=False, stop=True)
                nc.scalar.activation(h32[m][:], ps[:], AF.Relu)
            x5 = [keep.tile([128, SPC], dt.bfloat16, tag=f"x5_{m}", name=f"x5_{m}") for m in range(8)]
            for m in range(8):
                ps = pmisc.tile([128, 512], dt.float32, tag="pm")
                for kk in range(4):
                    nc.tensor.matmul(ps[:], W[f't3w3_{kk}{m}'][:], h32[kk][:],
                                     start=(kk == 0), stop=False)
                nc.tensor.matmul(ps[:], W[f't3w3_b{m}'][:], ones1[:], start=False, stop=True)
                mx = sp.tile([128, SPC], dt.float32, tag="mx")
                nc.vector.tensor_reduce(mx[:],
                                        ps[:].rearrange("p (s n) -> p s n", s=SPC),
                                        axis=mybir.AxisListType.X, op=AL.max)
                t1 = sp.tile([128, SPC], dt.float32, tag="t1x")
                nc.vector.tensor_scalar(t1[:], mx[:], 0.0, None, op0=AL.max)
                nc.scalar.activation(x5[m][:], t1[:], AF.Relu,
                                     bias=W[f'm3b_{m}'][:], scale=W[f'm3s_{m}'][:])
            # fc1
            x6 = [keep.tile([128, SPC], dt.bfloat16, tag=f"x6_{m}", name=f"x6_{m}") for m in range(2)]
            for m in range(2):
                ps = pmisc.tile([128, SPC], dt.float32, tag="pm")
                for kk in range(8):
                    nc.tensor.matmul(ps[:], W[f'fw1_{kk}{m}'][:], x5[kk][:],
                                     start=(kk == 0), stop=False)
                nc.tensor.matmul(ps[:], W[f'fw1_b{m}'][:], ones4[:], start=False, stop=True)
                nc.scalar.activation(x6[m][:], ps[:], AF.Relu)
            # fc2
            pso = pmisc.tile([60, SPC], dt.float32, tag="pm")
            nc.tensor.matmul(pso[:], W['fw2_0'][:], x6[0][:], start=True, stop=False)
            nc.tensor.matmul(pso[:], W['fw2_1'][:], x6[1][:], start=False, stop=False)
            nc.tensor.matmul(pso[:], W['fw2_b'][:], ones4[:], start=False, stop=True)
            outs = sp.tile([60, SPC], dt.float32, tag="outs")
            nc.scalar.activation(outs[:], pso[:], AF.Copy)
            nc.sync.dma_start(out_d[:], outs[:])
    nc.compile()
    return nc


def _get_built(debug=False):
    global _built
    key = bool(debug)
    if _built is None or _built[0] != key:
        _built = (key, build(debug=debug))
    return _built[1]


def kernel(xt, yt, params, debug=False, return_debug=False):
    from concourse.bass_utils import run_bass_kernel_spmd
    nc = _get_built(debug=debug)
    w = _prep_weights(params)
    xt = np.asarray(xt, np.float32).reshape(B, 6, N1 * K)
    # restage: [B, 8 rounds, 128 partitions, 512] with partition 32g+6sb+c = block 8r+2g+sb
    xts = np.zeros((B, 8, 128, 512), np.float32)
    blocks = xt.reshape(B, 6, 64, 512)                    # [B, c, block, n]
    for g in range(4):
        for sb in range(2):
            # partition rows 32g+6sb .. +6 get channels of block 8r+2g+sb for each round r
            xts[:, :, 32 * g + 6 * sb:32 * g + 6 * sb + 6, :] =                 blocks[:, :, 2 * g + sb::8, :].transpose(0, 2, 1, 3)
    xt_staged = xts.astype(BF)
    yt = np.asarray(yt, np.float32).reshape(B, 3, N1)
    in_maps = []
    for c in range(NCORE):
        m = dict(w)
        m['xt4'] = np.ascontiguousarray(xt_staged[SPC * c:SPC * c + SPC])
        m['yt4'] = np.ascontiguousarray(yt[SPC * c:SPC * c + SPC])
        in_maps.append(m)
    res = run_bass_kernel_spmd(nc, in_maps, core_ids=list(range(NCORE)))
    out = np.zeros((B, NUM_CLASS), np.float32)
    for c in range(NCORE):
        out[SPC * c:SPC * c + SPC] = res.results[c]['out'].T
    if return_debug:
        return out, res
    return out
